# revision 1
# baseline (speedup 1.0000x reference)
"""BotRGCN (2-layer relational GCN) Trainium2 kernel, 8-way SPMD.

Strategy (per sharding hint): nodes sharded contiguously across 8 cores;
edges partitioned by destination core; relation weights replicated.

Per core, per RGCN layer, for each 128-destination-node tile we accumulate
S^T[fi, c] (c = local_dst*4 + rel, rel 3 = root self-loop) in PSUM via
one-hot matmuls: gather source rows h[src] with SWDGE dma_gather (bf16,
256B rows), build narrow one-hot A matrices on DVE (iota==key)*norm, and
let the tensor engine do the scatter-add:  S^T += E^T @ A.  Then 4 weight
matmuls (3 relations + root) + bias produce the tile's output.  Feature
MLP (fc1/fc2/concat/leaky-relu) and the final fc3 also run on device; h
and h1 are AllGathered between layers so every core can gather any source
row.  The host only shards/packs inputs, plans edge slots, and indexes the
final logits with `idx`.

Self-contained: only imports the system concourse toolchain.
"""
import os
import sys

for _p in ("/opt/trn_rl_repo", "/root/.axon_site/_ro/trn_rl_repo"):
    if os.path.isdir(_p) and _p not in sys.path:
        sys.path.insert(0, _p)

import numpy as np
import ml_dtypes

from concourse import bass, bacc, tile, mybir
from concourse.bass_utils import run_bass_kernel_spmd

BF16 = ml_dtypes.bfloat16

# ---------------- problem constants (hardcoded per spec) ----------------
N_NODES = 50000
N_REL = 3
FEAT = 128
VAL = 16
TEXT = 768
CLASSES = 2
CORES = 8
P = 128           # partition / tile size
W = 64            # one-hot window width
CHMAX = 8         # slots per gather chunk (1024 idxs = SWDGE ring cap)
ABATCH = 32       # slots per A-matrix build batch
RSLOT = 4         # 3 relations + root self-loop


# ============================ host planner =============================

def _build_schedule(cts, cmax):
    """Joint (cross-core) slot schedule for one (tile, section).

    cts: list of 8 sorted int arrays (edge keys in [0, cmax)).
    Returns (bases, ranges) where bases[j] is the shared window base of
    slot j and ranges[c][j] = (start, end) into core c's sorted arrays.
    """
    n = len(cts)
    ptrs = [0] * n
    lens = [len(a) for a in cts]
    bases = []
    ranges = [[] for _ in range(n)]
    while any(ptrs[c] < lens[c] for c in range(n)):
        b = min(cts[c][ptrs[c]] for c in range(n) if ptrs[c] < lens[c])
        b = min(int(b), cmax - W)
        bases.append(b)
        for c in range(n):
            s = ptrs[c]
            hi = int(np.searchsorted(cts[c], b + W, side="left"))
            e = min(s + P, hi)
            e = max(e, s)
            ranges[c].append((s, e))
            ptrs[c] = e
    return bases, ranges


class Plan:
    pass


def make_plan(edge_index, edge_type, n_nodes=N_NODES, cores=CORES, lolim=None):
    """Edge partition + joint slot schedule shared by both RGCN layers."""
    pl = Plan()
    pl.cores = cores
    NS = n_nodes // cores
    assert NS * cores == n_nodes
    NSP = ((NS + P - 1) // P) * P
    NT = NSP // P
    NROWS = cores * NSP
    if lolim is None:
        lolim = min(NROWS, 32768)
    hibase = max(0, NROWS - 32768)
    assert hibase <= lolim  # coverage of both windows
    pl.NS, pl.NSP, pl.NT, pl.NROWS = NS, NSP, NT, NROWS
    pl.LOLIM, pl.HIBASE = lolim, hibase

    src = np.asarray(edge_index[0], np.int64)
    dst = np.asarray(edge_index[1], np.int64)
    et = np.asarray(edge_type, np.int64)

    deg = np.zeros((N_REL, n_nodes), np.int64)
    np.add.at(deg, (et, dst), 1)
    norm = 1.0 / np.maximum(deg[et, dst], 1).astype(np.float32)

    nodes = np.arange(n_nodes, dtype=np.int64)
    asrc = np.concatenate([src, nodes])
    adst = np.concatenate([dst, nodes])
    arel = np.concatenate([et, np.full(n_nodes, N_REL, np.int64)])
    anorm = np.concatenate([norm, np.ones(n_nodes, np.float32)])

    row = (asrc // NS) * NSP + (asrc % NS)
    owner = adst // NS
    loc = adst % NS
    tile_id = loc // P
    ct = (loc % P) * RSLOT + arel
    sec = (row >= lolim).astype(np.int64)

    order = np.lexsort((ct, sec, tile_id, owner))
    row, ct, sec, anorm = row[order], ct[order], sec[order], anorm[order]
    owner, tile_id = owner[order], tile_id[order]

    # index boundaries for (core, tile, sec) groups
    key = (owner * NT + tile_id) * 2 + sec
    bounds = np.searchsorted(key, np.arange(cores * NT * 2 + 1))

    def group(c, t, s):
        k = (c * NT + t) * 2 + s
        return bounds[k], bounds[k + 1]

    # per (tile, sec): joint schedule; accumulate per-core slot data
    slot_tile = {0: [], 1: []}      # per section stream: tile of each slot
    slot_base = {0: [], 1: []}
    idx16 = {0: [[] for _ in range(cores)], 1: [[] for _ in range(cores)]}
    keyd = {0: [[] for _ in range(cores)], 1: [[] for _ in range(cores)]}
    nrmd = {0: [[] for _ in range(cores)], 1: [[] for _ in range(cores)]}
    tile_slot_range = {0: np.zeros((NT, 2), np.int64), 1: np.zeros((NT, 2), np.int64)}

    for t in range(NT):
        for s in (0, 1):
            cts, rows_, nrms_ = [], [], []
            for c in range(cores):
                a, b = group(c, t, s)
                cts.append(ct[a:b])
                rows_.append(row[a:b])
                nrms_.append(anorm[a:b])
            start = len(slot_base[s])
            bases, ranges = _build_schedule(cts, P * RSLOT)
            for j, bj in enumerate(bases):
                slot_tile[s].append(t)
                slot_base[s].append(bj)
            for c in range(cores):
                for j, (a, b) in enumerate(ranges[c]):
                    n = b - a
                    ii = np.zeros(P, np.int16)
                    kk = np.full(P, -1.0, np.float32)
                    nn = np.zeros(P, np.float32)
                    r = rows_[c][a:b]
                    if s == 1:
                        r = r - hibase
                    ii[:n] = r.astype(np.int16)
                    kk[:n] = (cts[c][a:b] - bases[j]).astype(np.float32)
                    nn[:n] = nrms_[c][a:b]
                    idx16[s][c].append(ii)
                    keyd[s][c].append(kk)
                    nrmd[s][c].append(nn)
            tile_slot_range[s][t] = (start, len(slot_base[s]))

    pl.NLO = len(slot_base[0])
    pl.NHI = len(slot_base[1])
    pl.NSLOT = pl.NLO + pl.NHI
    pl.slot_base = {s: np.array(slot_base[s], np.int64) for s in (0, 1)}
    pl.slot_tile = {s: np.array(slot_tile[s], np.int64) for s in (0, 1)}
    pl.tile_slot_range = tile_slot_range

    # per-core packed arrays
    pl.idx_wrapped = {}
    pl.keys = {}
    pl.norms = {}
    for c in range(cores):
        parts = []
        for s in (0, 1):
            arr = (np.stack(idx16[s][c]) if idx16[s][c]
                   else np.zeros((0, P), np.int16))
            parts.append(arr)
        pl.idx_wrapped[c] = parts  # list of [nslot, 128] int16 per section
        kk = np.concatenate(
            [np.stack(keyd[s][c]) if keyd[s][c] else np.zeros((0, P), np.float32)
             for s in (0, 1)])
        nn = np.concatenate(
            [np.stack(nrmd[s][c]) if nrmd[s][c] else np.zeros((0, P), np.float32)
             for s in (0, 1)])
        pl.keys[c] = np.ascontiguousarray(kk.T.astype(BF16))    # [128, NSLOT]
        pl.norms[c] = np.ascontiguousarray(nn.T.astype(BF16))   # [128, NSLOT]

    # gather chunks per section stream: list of (s0, ns)
    pl.chunks = {}
    for s in (0, 1):
        n = [pl.NLO, pl.NHI][s]
        ch = []
        i = 0
        while i < n:
            ns = min(CHMAX, n - i)
            ch.append((i, ns))
            i += ns
        pl.chunks[s] = ch
    return pl


def wrap16(flat):
    """[L] int16 -> [128, L//16] wrapped layout for dma_gather idxs."""
    L = len(flat)
    assert L % 16 == 0
    a = np.asarray(flat, np.int16).reshape(-1, 16).T  # [16, L//16]
    return np.ascontiguousarray(np.tile(a, (8, 1)))



def blob_layout(pl):
    """Ordered (name, nelem, shape) segments of the single bf16 input blob.
    int16 segments are stored bit-cast as bf16. Offsets 128-elem aligned."""
    NSP, NT, NSLOT = pl.NSP, pl.NT, pl.NSLOT
    NLO, NHI = pl.NLO, pl.NHI
    TC = TEXT // P
    segs = [
        ("textT", [NT, P, TC * P]),
        ("valT", [VAL, NSP]),
        ("fc1w", [VAL, FEAT]),
        ("fc2w", [P, TC * P]),
        ("rwv", [FEAT, FEAT]),
        ("rwt", [FEAT, FEAT]),
        ("beff", [1, FEAT]),
        ("ww1", [P, RSLOT * FEAT]),
        ("b1", [1, FEAT]),
        ("ww2", [P, RSLOT * FEAT]),
        ("b2", [1, FEAT]),
        ("fc3w", [FEAT, CLASSES]),
        ("fc3b", [1, CLASSES]),
        ("iota", [P, W]),
        ("ones1", [1, P]),
        ("keys", [P, max(NSLOT, 1)]),
        ("norms", [P, max(NSLOT, 1)]),
        ("idxlo", [P, max(NLO, 1) * 8]),
        ("idxhi", [P, max(NHI, 1) * 8]),
    ]
    out = {}
    off = 0
    for name, shape in segs:
        n = int(np.prod(shape))
        out[name] = (off, n, shape)
        off += ((n + 127) // 128) * 128
    return out, off

# ============================ bass builder =============================

def build_bass(pl, ablate=()):
    ab = set(ablate)
    NSP, NT = pl.NSP, pl.NT
    NROWS = pl.NROWS
    NLO, NHI, NSLOT = pl.NLO, pl.NHI, pl.NSLOT
    TC = TEXT // P  # text chunks

    cores = getattr(pl, "cores", CORES)
    nc = bacc.Bacc("TRN2", target_bir_lowering=False, debug=False,
                   num_devices=cores, num_swdge_queues=4)
    qrr = {"n": 0}  # round-robin SWDGE queue picker
    dt = mybir.dt
    f32, bf, i16 = dt.float32, dt.bfloat16, dt.int16

    # ---- parameters: one packed bf16 blob + output
    layout, blob_n = blob_layout(pl)
    p_blob = nc.declare_dram_parameter("blob", [1, blob_n], bf, isOutput=False)
    p_logT = nc.declare_dram_parameter("logitsT", [CLASSES, NSP], f32, isOutput=True)

    def seg(name, dtype=bf):
        off, n, shape = layout[name]
        ap = p_blob[0:1, off:off + n]
        if dtype != bf:
            ap = ap.bitcast(dtype)
        r = int(np.prod(shape[:-1]))
        return ap.rearrange("o (r c) -> (o r) c", r=r)

    with tile.TileContext(nc) as tc:
        with tc.tile_pool(name="wt", bufs=1) as wt, \
             tc.tile_pool(name="sb", bufs=2) as sb, \
             tc.tile_pool(name="elo", bufs=10) as elo, \
             tc.tile_pool(name="ehi", bufs=10) as ehi, \
             tc.tile_pool(name="tts", bufs=3) as tts, \
             tc.tile_pool(name="dram", bufs=1, space="DRAM") as dram:

            # ---- resident weights / tables
            def resident(name, dtype=bf):
                off, n, shape = layout[name]
                t = wt.tile(list(shape[-2:] if len(shape) == 2 else shape), dtype,
                            tag=name)
                nc.sync.dma_start(t[:], seg(name, dtype))
                return t

            fc1w = resident("fc1w")
            fc2w = resident("fc2w")
            rwv = resident("rwv")
            rwt = resident("rwt")
            beff = resident("beff")
            ww1 = resident("ww1")
            b1 = resident("b1")
            ww2 = resident("ww2")
            b2 = resident("b2")
            fc3w = resident("fc3w")
            fc3b = resident("fc3b")
            iota = resident("iota")
            ones1 = resident("ones1")
            valT = resident("valT")
            keys = resident("keys")
            norms = resident("norms")
            idxsb = [resident("idxlo", i16), resident("idxhi", i16)]

            # ---- DRAM intermediates
            h_shard = dram.tile([NSP, FEAT], bf)
            _as = "Shared" if (cores > 1 and "coll" not in ab) else "Local"
            h_full = dram.tile([NROWS, FEAT], bf, addr_space=_as)
            h1_shard = dram.tile([NSP, FEAT], bf)
            h1_full = dram.tile([NROWS, FEAT], bf, addr_space=_as)

            # ================= phase 1: feature MLP =================
            with tc.tile_pool(name="ps1", bufs=2, space="PSUM") as ps1:
                for t in range(NT):
                    tt = tts.tile([P, TC, P], bf, tag="tt")
                    toff = layout["textT"][0] + t * P * TC * P
                    nc.sync.dma_start(
                        tt[:], p_blob[0:1, toff:toff + P * TC * P]
                        .rearrange("o (p c n) -> (o p) c n", p=P, c=TC))
                    pvT = ps1.tile([P, P], f32, tag="pvT", space="PSUM")
                    nc.tensor.matmul(out=pvT[:], lhsT=fc1w[:],
                                     rhs=valT[:, t * P:(t + 1) * P],
                                     start=True, stop=True)
                    vT = sb.tile([P, P], bf, tag="vT")
                    nc.vector.tensor_copy(out=vT[:], in_=pvT[:])
                    ptT = ps1.tile([P, P], f32, tag="ptT", space="PSUM")
                    for c in range(TC):
                        nc.tensor.matmul(out=ptT[:],
                                         lhsT=fc2w[:, c * P:(c + 1) * P],
                                         rhs=tt[:, c, :],
                                         start=(c == 0), stop=(c == TC - 1))
                    tT = sb.tile([P, P], bf, tag="tT")
                    nc.vector.tensor_copy(out=tT[:], in_=ptT[:])
                    ph = ps1.tile([P, P], f32, tag="ph", space="PSUM")
                    nc.tensor.matmul(out=ph[:], lhsT=vT[:], rhs=rwv[:],
                                     start=True, stop=False)
                    nc.tensor.matmul(out=ph[:], lhsT=tT[:], rhs=rwt[:],
                                     start=False, stop=False)
                    nc.tensor.matmul(out=ph[:], lhsT=ones1[:], rhs=beff[:],
                                     start=False, stop=True)
                    hsb = sb.tile([P, P], bf, tag="hsb")
                    lk = sb.tile([P, P], f32, tag="lk")
                    nc.vector.tensor_scalar(out=lk[:], in0=ph[:], scalar1=0.01,
                                            scalar2=None, op0=mybir.AluOpType.mult)
                    nc.vector.tensor_tensor(out=hsb[:], in0=ph[:], in1=lk[:],
                                            op=mybir.AluOpType.max)
                    nc.sync.dma_start(h_shard[t * P:(t + 1) * P, :], hsb[:])

            if cores > 1 and "coll" not in ab:
                nc.gpsimd.collective_compute(
                    "AllGather", mybir.AluOpType.bypass,
                    replica_groups=[list(range(cores))],
                    ins=[h_shard.opt()], outs=[h_full.opt()])
            else:
                nc.sync.dma_start(h_full[0:NSP, :], h_shard[:])

            # ================= RGCN layers =================
            def rgcn_layer(src_full, ww, bb, layer, out_shard):
                emitted = {0: -1, 1: -1}   # last emitted gather chunk per stream
                aemitted = {0: -1, 1: -1}  # last emitted A batch per stream
                ebufs = {0: {}, 1: {}}     # chunk id -> (E tile, s0, ns)
                abufs = {0: {}, 1: {}}     # batch id -> (A tile, s0, ns)
                pools = {0: elo, 1: ehi}
                nstream = {0: NLO, 1: NHI}

                def emit_chunk(s, ci):
                    s0 = ci * CHMAX
                    ns = min(CHMAX, nstream[s] - s0)
                    et = pools[s].tile([P, CHMAX, FEAT], bf, tag=f"e{s}")
                    if s == 0:
                        src_ap = src_full[0:pl.LOLIM, :]
                    else:
                        src_ap = src_full[pl.HIBASE:pl.NROWS, :]
                    if "gather" in ab:
                        nc.vector.memset(et[:, 0:1, 0:2], 0.0)
                        ebufs[s][ci] = (et, s0, ns)
                        ebufs[s].pop(ci - 9, None)
                        return
                    qrr["n"] += 1
                    nc.gpsimd.dma_gather(
                        out_ap=et[:, 0:ns, :],
                        in_ap=src_ap,
                        idxs_ap=idxsb[s][:, s0 * 8:(s0 + ns) * 8],
                        num_idxs=ns * P,
                        num_idxs_reg=ns * P,
                        elem_size=FEAT,
                        queue_num=qrr["n"] % 4)
                    ebufs[s][ci] = (et, s0, ns)
                    ebufs[s].pop(ci - 9, None)

                def emit_abatch(s, ai):
                    s0 = ai * ABATCH
                    ns = min(ABATCH, nstream[s] - s0)
                    at = pools[s].tile([P, ABATCH, W], bf, tag=f"a{s}")
                    if "abuild" in ab:
                        nc.vector.memset(at[:, 0:1, 0:2], 0.0)
                        abufs[s][ai] = (at, s0, ns)
                        abufs[s].pop(ai - 3, None)
                        return
                    g0 = s0 + (0 if s == 0 else NLO)
                    kb = keys[:, g0:g0 + ns].unsqueeze(2).to_broadcast([P, ns, W])
                    nb = norms[:, g0:g0 + ns].unsqueeze(2).to_broadcast([P, ns, W])
                    ib = iota[:].unsqueeze(1).to_broadcast([P, ns, W])
                    nc.vector.tensor_tensor(out=at[:, 0:ns, :], in0=ib, in1=kb,
                                            op=mybir.AluOpType.is_equal)
                    nc.vector.tensor_tensor(out=at[:, 0:ns, :], in0=at[:, 0:ns, :],
                                            in1=nb, op=mybir.AluOpType.mult)
                    abufs[s][ai] = (at, s0, ns)
                    abufs[s].pop(ai - 3, None)

                with tc.tile_pool(name=f"psl{layer}", bufs=2, space="PSUM") as psl:
                    for t in range(NT):
                        pS = psl.tile([P, RSLOT * P], f32, tag="pS", space="PSUM")
                        if "memset" not in ab:
                            nc.vector.memset(pS[:], 0.0)
                        for s in (0, 1):
                            a, b = pl.tile_slot_range[s][t]
                            for j in range(a, b):
                                ci = j // CHMAX
                                ai = j // ABATCH
                                if ci > emitted[s]:
                                    emit_chunk(s, ci)
                                    emitted[s] = ci
                                if ai > aemitted[s]:
                                    emit_abatch(s, ai)
                                    aemitted[s] = ai
                                et, es0, _ = ebufs[s][ci]
                                at, as0, _ = abufs[s][ai]
                                bj = int(pl.slot_base[s][j])
                                if "slotmm" in ab:
                                    continue
                                nc.tensor.matmul(
                                    out=pS[:, bj:bj + W],
                                    lhsT=et[:, j - es0, :], rhs=at[:, j - as0, :],
                                    start=False, stop=False,
                                    skip_group_check=True)
                        sS = sb.tile([P, RSLOT * P], bf, tag="sS")
                        nc.scalar.activation(out=sS[:], in_=pS[:],
                                             func=mybir.ActivationFunctionType.Copy)
                        if layer == 1:
                            pO = psl.tile([P, FEAT], f32, tag="pO", space="PSUM")
                            for r in range(RSLOT):
                                nc.tensor.matmul(out=pO[:], lhsT=sS[:, r::RSLOT],
                                                 rhs=ww[:, r * FEAT:(r + 1) * FEAT],
                                                 start=(r == 0), stop=False)
                            nc.tensor.matmul(out=pO[:], lhsT=ones1[:], rhs=bb[:],
                                             start=False, stop=True)
                            ho = sb.tile([P, FEAT], bf, tag="ho")
                            nc.vector.tensor_copy(out=ho[:], in_=pO[:])
                            nc.sync.dma_start(out_shard[t * P:(t + 1) * P, :], ho[:])
                        else:
                            pO = psl.tile([P, P], f32, tag="pO", space="PSUM")
                            for r in range(RSLOT):
                                nc.tensor.matmul(out=pO[:],
                                                 lhsT=ww[:, r * FEAT:(r + 1) * FEAT],
                                                 rhs=sS[:, r::RSLOT],
                                                 start=(r == 0), stop=False)
                            nc.tensor.matmul(out=pO[:], lhsT=b2[:], rhs=ones1[:],
                                             start=False, stop=True)
                            h2T = sb.tile([P, P], bf, tag="h2T")
                            nc.vector.tensor_copy(out=h2T[:], in_=pO[:])
                            pL = psl.tile([CLASSES, P], f32, tag="pL", space="PSUM")
                            nc.tensor.matmul(out=pL[:], lhsT=fc3w[:], rhs=h2T[:],
                                             start=True, stop=False)
                            nc.tensor.matmul(out=pL[:], lhsT=fc3b[:], rhs=ones1[:],
                                             start=False, stop=True)
                            lg = sb.tile([CLASSES, P], f32, tag="lg")
                            nc.vector.tensor_copy(out=lg[:], in_=pL[:])
                            nc.sync.dma_start(p_logT[:, t * P:(t + 1) * P], lg[:])

            rgcn_layer(h_full, ww1, b1, 1, h1_shard)
            if cores > 1 and "coll" not in ab:
                nc.gpsimd.collective_compute(
                    "AllGather", mybir.AluOpType.bypass,
                    replica_groups=[list(range(cores))],
                    ins=[h1_shard.opt()], outs=[h1_full.opt()])
            else:
                nc.sync.dma_start(h1_full[0:NSP, :], h1_shard[:])
            rgcn_layer(h1_full, ww2, b2, 2, None)

    nc.compile()
    return nc


# ============================ host packing =============================

def pack_inputs(pl, inputs):
    """Build per-core in_maps from the full problem inputs."""
    NS, NSP, NT = pl.NS, pl.NSP, pl.NT
    TC = TEXT // P

    vf = np.asarray(inputs["value_feature"], np.float32)
    tf = np.asarray(inputs["text_feature"], np.float32)
    n = vf.shape[0]

    def shard_textT(c):
        x = np.zeros((NSP, TEXT), np.float32)
        x[:NS] = tf[c * NS:(c + 1) * NS]
        # [NT, 128p(k within chunk), TC, 128n] -> flat [NT, 128, TC*128]
        y = x.reshape(NT, P, TC, P).transpose(0, 3, 2, 1)
        return np.ascontiguousarray(y.reshape(NT, P, TC * P).astype(BF16))

    def shard_valT(c):
        x = np.zeros((NSP, VAL), np.float32)
        x[:NS] = vf[c * NS:(c + 1) * NS]
        return np.ascontiguousarray(x.T.astype(BF16))

    f32 = np.float32
    fc1w = np.asarray(inputs["fc1_w"], f32)
    fc2w = np.asarray(inputs["fc2_w"], f32)
    relw = np.asarray(inputs["relu_w"], f32)
    beff = (np.concatenate([np.asarray(inputs["fc1_b"], f32),
                            np.asarray(inputs["fc2_b"], f32)]) @ relw
            + np.asarray(inputs["relu_b"], f32))
    # fc2w host layout [128 k, TC*128 f]: [k, c*128+f] = fc2_w[c*128+k, f]
    fc2w_t = np.ascontiguousarray(
        fc2w.reshape(TC, P, FEAT).transpose(1, 0, 2).reshape(P, TC * FEAT).astype(BF16))

    def stack_w(wrel, wroot):
        w = np.concatenate([np.asarray(wrel, f32),
                            np.asarray(wroot, f32)[None]], 0)  # [4,128,128]
        return np.ascontiguousarray(w.transpose(1, 0, 2).reshape(P, RSLOT * FEAT).astype(BF16))

    ww1 = stack_w(inputs["rgcn1_wrel"], inputs["rgcn1_wroot"])
    ww2 = stack_w(inputs["rgcn2_wrel"], inputs["rgcn2_wroot"])

    iota = np.tile(np.arange(W, dtype=f32), (P, 1)).astype(BF16)
    ones1 = np.ones((1, P), f32).astype(BF16)

    layout, blob_n = blob_layout(pl)
    shared = dict(
        fc1w=fc1w.astype(BF16), fc2w=fc2w_t,
        rwv=np.ascontiguousarray(relw[:FEAT].astype(BF16)),
        rwt=np.ascontiguousarray(relw[FEAT:].astype(BF16)),
        beff=beff[None].astype(BF16),
        ww1=ww1, b1=np.asarray(inputs["rgcn1_b"], f32)[None].astype(BF16),
        ww2=ww2, b2=np.asarray(inputs["rgcn2_b"], f32)[None].astype(BF16),
        fc3w=np.asarray(inputs["fc3_w"], f32).astype(BF16),
        fc3b=np.asarray(inputs["fc3_b"], f32)[None].astype(BF16),
        iota=np.tile(np.arange(W, dtype=f32), (P, 1)).astype(BF16),
        ones1=np.ones((1, P), f32).astype(BF16),
    )

    in_maps = []
    for c in range(CORES):
        lo, hi = pl.idx_wrapped[c]
        vals = dict(shared)
        vals["textT"] = shard_textT(c)
        vals["valT"] = shard_valT(c)
        vals["idxlo"] = (wrap16(lo.reshape(-1)) if lo.size
                         else np.zeros((P, 8), np.int16)).view(BF16)
        vals["idxhi"] = (wrap16(hi.reshape(-1)) if hi.size
                         else np.zeros((P, 8), np.int16)).view(BF16)
        vals["keys"] = pl.keys[c] if pl.NSLOT else np.zeros((P, 1), BF16)
        vals["norms"] = pl.norms[c] if pl.NSLOT else np.zeros((P, 1), BF16)
        blob = np.zeros((1, blob_n), BF16)
        for name, (off, n, shape) in layout.items():
            a = vals[name]
            assert a.size == n, (name, a.shape, shape)
            blob[0, off:off + n] = a.reshape(-1)
        in_maps.append({"blob": blob})
    return in_maps


# ============================ entry point =============================

_cache = {}


def kernel(**inputs):
    ei = np.asarray(inputs["edge_index"], np.int64)
    et = np.asarray(inputs["edge_type"], np.int64)
    idx = np.asarray(inputs["idx"], np.int64)

    key = hash((ei.tobytes(), et.tobytes()))
    if key not in _cache:
        pl = make_plan(ei, et)
        nc = build_bass(pl)
        _cache[key] = (pl, nc)
    pl, nc = _cache[key]

    in_maps = pack_inputs(pl, inputs)
    res = run_bass_kernel_spmd(nc, in_maps, list(range(CORES)))

    NS, NSP = pl.NS, pl.NSP
    logits = np.zeros((N_NODES, CLASSES), np.float32)
    for c in range(CORES):
        lt = res.results[c]["logitsT"]  # [2, NSP]
        logits[c * NS:(c + 1) * NS] = lt[:, :NS].T
    out = logits[idx]
    return out.astype(np.float32)

